# revision 3
# baseline (speedup 1.0000x reference)
"""Trainium2 Bass kernel for LocalRelationalLayer — v2 (engine-balanced).

Computation (per reference):
  xp = zero-pad(x, 3)                                   # [B,256,62,62]
  km = 1x1conv(xp, k_w)+k_b ; qm = 1x1conv(xp, q_w)+q_b # [B,32,·,·]
  E[b,cm,l,ky,kx] = exp(km[b,cm,r+ky,w+kx]*qm[b,cm,r+3,w+3] + gpk[cm,ky,kx])
  ck = E / sum_kx E                                     # softmax over kx only
  pre[b,m*32+cm,l] = sum_{ky,kx} ck * xp[b,m*32+cm,r+ky,w+kx]
  out = 1x1conv(pre, f_w)+f_b                           # [B,256,56,56]

Sharding: 8 cores = (b in 2) x (4 row-blocks of 14 output rows); core-local,
no collectives (halo rows included per core).

v2 engine plan (cost-model-driven):
  - ACT: only the 49 exp ops (bias = gpk per (ky,kx)); no table switches.
  - PE: d-sums and S-sums as identity-stationary PSUM-accumulating matmuls,
    plus the final 1x1 conv.
  - DVE/Pool: the P/PV multiplies and the softmax divide, statically split.
  - Softmax scale via tensor_tensor divide (T = S/d) -- no Ln/Exp reciprocal.
"""

import numpy as np
import ml_dtypes

B, C, H, W = 2, 256, 56, 56
K, PAD, M, CM = 7, 3, 8, 32
HP, WP = H + 2 * PAD, W + 2 * PAD      # 62, 62
RB = 4                                  # row blocks per batch
RH = H // RB                            # 14 output rows per core
RHP = RH + K - 1                        # 20 padded rows per core
NCORES = 8
L = RH * W                              # 784 output positions per core
HL = L // 2                             # 392 (contiguous r-half)

_bf16 = ml_dtypes.bfloat16
_PROGRAM = None

# --- variant toggles (tuned via CoreSim) ---
CFG = {
    "d_on_pe": True,       # d-sums via PE identity-accum (else engine tree)
    "s_on_pe": 2,          # how many of the 2 chunks' S-sums go to PE (0/1/2)
    "conv_per_ky": False,  # accumulate final conv per ky in PSUM (else at end)
    "pv_pool": 5,          # of the 14 PV muls per ky, how many go to Pool
    "p_pool": 0,           # of the 7 P muls per ky, how many go to Pool
}


def _build_program(cfg=None):
    import concourse.bass as bass
    import concourse.tile as tile
    from concourse import bacc, mybir
    import bass_rust

    def win_view(tile_obj, ky):
        """Overlapping sliding-window view [128, RH, W, K] of a
        [128, RHP, WP] tile: elem (p, r, w, kx) -> tile[p, ky+r, w+kx]."""
        ap = tile_obj[:]
        v = ap.copy()
        v.ap = bass_rust.VecI64Pair(
            [[RHP * WP, 128], [WP, RH], [1, W], [1, K]])
        v.offset = ky * WP
        return v

    cfg = dict(CFG if cfg is None else cfg)
    f32 = mybir.dt.float32
    bf16 = mybir.dt.bfloat16
    Exp = mybir.ActivationFunctionType.Exp
    Ident = mybir.ActivationFunctionType.Identity
    Div = mybir.AluOpType.divide
    PS = bass.MemorySpace.PSUM

    nc = bacc.Bacc("TRN2", target_bir_lowering=False, debug=False,
                   num_devices=NCORES)

    xp_d = nc.dram_tensor("xp", [2, 128, RHP * WP], bf16, kind="ExternalInput")
    # packed weights: [wq(2*128) | wk(2*128) | fw(4*128) | ident(128)] bf16
    wpk_d = nc.dram_tensor("wpk", [128, 9 * 128], bf16, kind="ExternalInput")
    # packed scalars: [qb | kb | gpk(49) | fb(2)] f32
    spk_d = nc.dram_tensor("spk", [128, 53], f32, kind="ExternalInput")
    y_d = nc.dram_tensor("y", [2, 128, RH * W], f32, kind="ExternalOutput")

    with tile.TileContext(nc) as tc:
        with (
            tc.tile_pool(name="inp", bufs=1) as inp,
            tc.tile_pool(name="wpool", bufs=1) as wpool,
            tc.tile_pool(name="kq", bufs=1) as kq,
            tc.tile_pool(name="pp", bufs=2) as pp,
            tc.tile_pool(name="ew", bufs=(K if cfg.get("phased") else 4)) as ew,
            tc.tile_pool(name="dd", bufs=(K if cfg.get("phased") else 2)) as dd,
            tc.tile_pool(name="pv", bufs=4) as pvp,
            tc.tile_pool(name="sm", bufs=2) as sm,
            tc.tile_pool(name="outp", bufs=1) as outp,
            tc.tile_pool(name="psA", bufs=2, space=PS) as psA,
            tc.tile_pool(name="psD", bufs=1, space=PS) as psD,
            tc.tile_pool(name="psS", bufs=2, space=PS) as psS,
        ):
            # ---- load inputs (one packed DMA for all weights) ----
            xv = []
            for c2 in range(2):
                t = inp.tile([128, RHP, WP], bf16, tag=f"xv{c2}", name=f"xv{c2}")
                nc.sync.dma_start(t[:].rearrange("p r w -> p (r w)"), xp_d.ap()[c2])
                xv.append(t)
            wpk = wpool.tile([128, 9, 128], bf16, tag="wpk", name="wpk")
            nc.sync.dma_start(wpk[:].rearrange("p a b -> p (a b)"), wpk_d.ap())
            spk = wpool.tile([128, 53], f32, tag="spk", name="spk")
            nc.sync.dma_start(spk[:], spk_d.ap())
            wq = [wpk[:, 0], wpk[:, 1]]
            wk = [wpk[:, 2], wpk[:, 3]]
            fw = [[wpk[:, 4], wpk[:, 5]], [wpk[:, 6], wpk[:, 7]]]
            ident = wpk[:, 8]
            qb = spk[:, 0:1]
            kb = spk[:, 1:2]
            gpk = spk[:, 2:51]
            fb = [spk[:, 51:52], spk[:, 52:53]]
            # ---- warmup: ramp the PE p-state and load the ACT table while
            # the input DMAs are in flight (scratch data, results unused) ----
            warm = wpool.tile([128, 64], bf16, tag="warm", name="warm")
            nc.vector.memset(warm[:], 0.0)
            wps = psA.tile([128, 512], f32, tag="ps", name="warmps")
            for i in range(40):
                nc.tensor.matmul(wps[0:64, 0:64], warm[:], warm[:],
                                 start=True, stop=True)
            wact = wpool.tile([128, 1], f32, tag="wact", name="wact")
            nc.scalar.activation(wact[:], warm[:, 0:1], Exp, bias=0.0,
                                 scale=1.0)

            # ---- qm (center rows) first: it gates qmc7 and every P ----
            qm = kq.tile([128, RH, WP], bf16, tag="qm", name="qm")
            qm_f = qm[:].rearrange("p r w -> p (r w)")
            NQM = RH * WP  # 868
            for off in range(0, NQM, 496):
                n = min(496, NQM - off)
                ps = psA.tile([128, 512], f32, tag="ps", name="ps")
                for c2 in range(2):
                    rhs = xv[c2][:].rearrange("p r w -> p (r w)")[:, PAD * WP + off:
                                                                 PAD * WP + off + n]
                    nc.tensor.matmul(ps[:, :n], wq[c2], rhs,
                                     start=(c2 == 0), stop=(c2 == 1))
                nc.scalar.activation(qm_f[:, off:off + n], ps[:, :n], Ident,
                                     bias=qb, scale=1.0)
            km = kq.tile([128, RHP, WP], bf16, tag="km", name="km")
            km_f = km[:].rearrange("p r w -> p (r w)")
            NKM = RHP * WP  # 1240
            for off in range(0, NKM, 496):
                n = min(496, NKM - off)
                ps = psA.tile([128, 512], f32, tag="ps", name="ps")
                for c2 in range(2):
                    rhs = xv[c2][:].rearrange("p r w -> p (r w)")[:, off:off + n]
                    nc.tensor.matmul(ps[:, :n], wk[c2], rhs,
                                     start=(c2 == 0), stop=(c2 == 1))
                nc.scalar.activation(km_f[:, off:off + n], ps[:, :n], Ident,
                                     bias=kb, scale=1.0)
            qmc = qm[:, :, PAD:PAD + W]   # [128, 14, 56]
            # one-time: qm center replicated over kx (kx innermost) for the
            # single-op fused P multiply
            qmc7 = kq.tile([128, RH, W, K], bf16, tag="qmc7", name="qmc7")
            for (k0, k1), eng in (((0, 2), nc.gpsimd), ((2, 4), nc.vector),
                                  ((4, 6), nc.gpsimd), ((6, 7), nc.vector)):
                qmc_b = qmc.copy()
                qmc_b.ap = bass_rust.VecI64Pair(
                    [[RH * WP, 128], [WP, RH], [1, W], [0, k1 - k0]])
                qmc_b.offset = PAD
                if eng is nc.scalar:
                    eng.copy(qmc7[:, :, :, k0:k1], qmc_b)
                else:
                    eng.tensor_copy(qmc7[:, :, :, k0:k1], qmc_b)

            pre = [outp.tile([128, RH, W], bf16, tag=f"pre{c2}", name=f"pre{c2}")
                   for c2 in range(2)]
            t6 = [outp.tile([128, RH, W], bf16, tag=f"t6{c2}", name=f"t6{c2}")
                  for c2 in range(2)]

            # ---- main loop: phase 1 per ky computes P, E, d (score side);
            # phase 2 per (ky, c2) does PV -> S (PE) -> T = S/d (value side).
            # In phased mode all score work is emitted first so the event-
            # driven engines stay leveled; E/d tiles persist per ky. ----
            Es, ds = [], []

            def score_phase(ky):
                P = pp.tile([128, RH, W, K], bf16, tag="P", name="P")
                kv = win_view(km, ky)
                nc.vector.tensor_mul(P[:, :, :, 0:4], kv[:, :, :, 0:4],
                                     qmc7[:, :, :, 0:4])
                nc.vector.tensor_mul(P[:, :, :, 4:K], kv[:, :, :, 4:K],
                                     qmc7[:, :, :, 4:K])
                E = ew.tile([128, RH, W, K], bf16, tag="E", name=f"E{ky}")
                for kx in range(K):
                    j = ky * K + kx
                    nc.scalar.activation(E[:, :, :, kx], P[:, :, :, kx], Exp,
                                         bias=spk[:, 2 + j:3 + j], scale=1.0)
                r = dd.tile([128, RH, W], f32, tag="r", name=f"r{ky}")
                dps = psD.tile([128, 2, 512], f32, tag="dps", name="dps")
                for kx in range(K):
                    nc.tensor.matmul(dps[:, 0, 0:HL], ident,
                                     E[:, 0:RH // 2, :, kx],
                                     start=(kx == 0), stop=(kx == K - 1))
                    nc.tensor.matmul(dps[:, 1, 0:HL], ident,
                                     E[:, RH // 2:RH, :, kx],
                                     start=(kx == 0), stop=(kx == K - 1))
                r_f = r[:].rearrange("p r w -> p (r w)")
                nc.vector.reciprocal_approx_fast(
                    r_f[:].rearrange("p (h x) -> p h x", h=2),
                    dps[:, :, 0:HL])
                return E, r

            def value_phase(ky, c2, E, r):
                r_f = r[:].rearrange("p r w -> p (r w)")
                PV = pvp.tile([128, RH, W, K], bf16, tag=f"PV{c2}",
                              name=f"PV{c2}")
                xvv = win_view(xv[c2], ky)
                peng = nc.vector if (c2 == 0 or ky == K - 1) else nc.gpsimd
                peng.tensor_mul(PV[:, :, :, 0:4], E[:, :, :, 0:4],
                                xvv[:, :, :, 0:4])
                nc.gpsimd.tensor_mul(PV[:, :, :, 4:K], E[:, :, :, 4:K],
                                     xvv[:, :, :, 4:K])
                sp = psS.tile([128, 2, 512], f32, tag="sp", name="sp")
                for kx in range(K):
                    nc.tensor.matmul(sp[:, 0, 0:HL], ident,
                                     PV[:, 0:RH // 2, :, kx],
                                     start=(kx == 0), stop=(kx == K - 1))
                    nc.tensor.matmul(sp[:, 1, 0:HL], ident,
                                     PV[:, RH // 2:RH, :, kx],
                                     start=(kx == 0), stop=(kx == K - 1))
                dst = pre[c2] if ky == 0 else (
                    t6[c2] if ky == K - 1 else
                    sm.tile([128, RH, W], bf16, tag=f"T{c2}", name=f"T{c2}"))
                dst_f = dst[:].rearrange("p r w -> p (r w)")
                nc.vector.tensor_mul(
                    dst_f[:].rearrange("p (h x) -> p h x", h=2),
                    sp[:, :, 0:HL],
                    r_f[:].rearrange("p (h x) -> p h x", h=2))
                if 0 < ky < K - 1:
                    nc.gpsimd.tensor_add(pre[c2][:], pre[c2][:], dst[:])

            if cfg.get("phased"):
                for ky in range(K):
                    Es_ds = score_phase(ky)
                    Es.append(Es_ds[0]); ds.append(Es_ds[1])
                for ky in range(K):
                    for c2 in range(2):
                        value_phase(ky, c2, Es[ky], ds[ky])
            else:
                for ky in range(K):
                    E, d = score_phase(ky)
                    for c2 in range(2):
                        value_phase(ky, c2, E, d)

            # ---- final 1x1 conv: psum accumulates fw^T pre (ky0..5, can
            # run during ky6) then fw^T t6 closes it ----
            for o in range(2):
                y_sb = outp.tile([128, RH, W], f32, tag=f"y{o}", name=f"y{o}")
                y_f = y_sb[:].rearrange("p r w -> p (r w)")
                for h in range(2):
                    ps = psA.tile([128, 512], f32, tag="ps", name="psf")
                    for ci in range(2):
                        rhs = pre[ci][:].rearrange("p r w -> p (r w)")[
                            :, h * HL:(h + 1) * HL]
                        nc.tensor.matmul(ps[:, :HL], fw[ci][o], rhs,
                                         start=(ci == 0), stop=False)
                    for ci in range(2):
                        rhs = t6[ci][:].rearrange("p r w -> p (r w)")[
                            :, h * HL:(h + 1) * HL]
                        nc.tensor.matmul(ps[:, :HL], fw[ci][o], rhs,
                                         start=False, stop=(ci == 1))
                    nc.scalar.activation(y_f[:, h * HL:(h + 1) * HL],
                                         ps[:, :HL], Ident,
                                         bias=fb[o], scale=1.0)
                    nc.sync.dma_start(y_d.ap()[o][:, h * HL:(h + 1) * HL],
                                      y_f[:, h * HL:(h + 1) * HL])

    nc.compile()
    return nc


# revision 4
# speedup vs baseline: 1.0062x; 1.0062x over previous
"""Trainium2 Bass kernel for LocalRelationalLayer (sparse_attention).

Computation (per reference):
  xp = zero-pad(x, 3)                                   # [B,256,62,62]
  km = 1x1conv(xp, k_w)+k_b ; qm = 1x1conv(xp, q_w)+q_b # [B,32,·,·]
  E[b,cm,l,ky,kx] = exp(km[b,cm,r+ky,w+kx]*qm[b,cm,r+3,w+3] + gpk[cm,ky,kx])
  ck = E / sum_kx E                                     # softmax over kx only
  pre[b,m*32+cm,l] = sum_{ky,kx} ck * xp[b,m*32+cm,r+ky,w+kx]
  out = 1x1conv(pre, f_w)+f_b                           # [B,256,56,56]

Sharding: 8 cores = (b in 2) x (4 row-blocks of 14 output rows); every step is
core-local (3-row halo in the per-core input slice), host concatenates.

Engine plan (all four compute engines balanced, cost-model-driven):
  - PE: km/qm/final 1x1 convs, plus BOTH window reductions (sum_kx E and
    sum_kx E*xv) as identity-stationary PSUM-accumulating matmuls over
    kx-strided slices (kx is the innermost free dim of E/PV).
  - ACT: only the 49 exp ops (gpk rides the per-partition bias; Identity,
    Copy and Exp share one activation table -> a single table load).
  - DVE: fused single-op P = km*qm (overlapping sliding-window AP against a
    one-time kx-replicated qm tile), half the PV = E*xv multiplies,
    r = reciprocal_approx_fast(d) straight from PSUM, T = S_psum * r.
  - Pool (no PSUM access on real HW): the other PV multiplies and the
    pre += T accumulations.
Key layout: [128, r, w, kx] with kx innermost keeps every DVE multiply in the
2x bf16 perf mode (stride-1 innermost, broadcasts only on middle dims).
Extras: PE p-state + ACT-table warmup ops behind the input DMAs; packed
single-DMA weights; eager final conv (psum takes ky0..5 during ky6).
"""

import numpy as np
import ml_dtypes

B, C, H, W = 2, 256, 56, 56
K, PAD, M, CM = 7, 3, 8, 32
HP, WP = H + 2 * PAD, W + 2 * PAD      # 62, 62
RB = 4                                  # row blocks per batch
RH = H // RB                            # 14 output rows per core
RHP = RH + K - 1                        # 20 padded rows per core
NCORES = 8
L = RH * W                              # 784 output positions per core
HL = L // 2                             # 392 (contiguous r-half)

_bf16 = ml_dtypes.bfloat16
_PROGRAM = None

# --- variant toggles (tuned via CoreSim) ---
CFG = {
    "d_on_pe": True,       # d-sums via PE identity-accum (else engine tree)
    "s_on_pe": 2,          # how many of the 2 chunks' S-sums go to PE (0/1/2)
    "conv_per_ky": False,  # accumulate final conv per ky in PSUM (else at end)
    "pv_pool": 5,          # of the 14 PV muls per ky, how many go to Pool
    "p_pool": 0,           # of the 7 P muls per ky, how many go to Pool
}


def _build_program(cfg=None):
    import concourse.bass as bass
    import concourse.tile as tile
    from concourse import bacc, mybir
    import bass_rust

    def win_view(tile_obj, ky):
        """Overlapping sliding-window view [128, RH, W, K] of a
        [128, RHP, WP] tile: elem (p, r, w, kx) -> tile[p, ky+r, w+kx]."""
        ap = tile_obj[:]
        v = ap.copy()
        v.ap = bass_rust.VecI64Pair(
            [[RHP * WP, 128], [WP, RH], [1, W], [1, K]])
        v.offset = ky * WP
        return v

    cfg = dict(CFG if cfg is None else cfg)
    f32 = mybir.dt.float32
    bf16 = mybir.dt.bfloat16
    Exp = mybir.ActivationFunctionType.Exp
    Ident = mybir.ActivationFunctionType.Identity
    Div = mybir.AluOpType.divide
    PS = bass.MemorySpace.PSUM

    nc = bacc.Bacc("TRN2", target_bir_lowering=False, debug=False,
                   num_devices=NCORES)

    xp_d = nc.dram_tensor("xp", [2, 128, RHP * WP], bf16, kind="ExternalInput")
    # packed weights: [wq(2*128) | wk(2*128) | fw(4*128) | ident(128)] bf16
    wpk_d = nc.dram_tensor("wpk", [128, 9 * 128], bf16, kind="ExternalInput")
    # packed scalars: [qb | kb | gpk(49) | fb(2)] f32
    spk_d = nc.dram_tensor("spk", [128, 53], f32, kind="ExternalInput")
    y_d = nc.dram_tensor("y", [2, 128, RH * W], f32, kind="ExternalOutput")

    with tile.TileContext(nc) as tc:
        with (
            tc.tile_pool(name="inp", bufs=1) as inp,
            tc.tile_pool(name="wpool", bufs=1) as wpool,
            tc.tile_pool(name="kq", bufs=1) as kq,
            tc.tile_pool(name="pp", bufs=2) as pp,
            tc.tile_pool(name="ew", bufs=(K if cfg.get("phased") else 4)) as ew,
            tc.tile_pool(name="dd", bufs=(K if cfg.get("phased") else 2)) as dd,
            tc.tile_pool(name="pv", bufs=4) as pvp,
            tc.tile_pool(name="sm", bufs=2) as sm,
            tc.tile_pool(name="outp", bufs=1) as outp,
            tc.tile_pool(name="psA", bufs=2, space=PS) as psA,
            tc.tile_pool(name="psD", bufs=1, space=PS) as psD,
            tc.tile_pool(name="psS", bufs=2, space=PS) as psS,
        ):
            # ---- load inputs (one packed DMA for all weights) ----
            xv = []
            for c2 in range(2):
                t = inp.tile([128, RHP, WP], bf16, tag=f"xv{c2}", name=f"xv{c2}")
                nc.sync.dma_start(t[:].rearrange("p r w -> p (r w)"), xp_d.ap()[c2])
                xv.append(t)
            wpk = wpool.tile([128, 9, 128], bf16, tag="wpk", name="wpk")
            nc.sync.dma_start(wpk[:].rearrange("p a b -> p (a b)"), wpk_d.ap())
            spk = wpool.tile([128, 53], f32, tag="spk", name="spk")
            nc.sync.dma_start(spk[:], spk_d.ap())
            wq = [wpk[:, 0], wpk[:, 1]]
            wk = [wpk[:, 2], wpk[:, 3]]
            fw = [[wpk[:, 4], wpk[:, 5]], [wpk[:, 6], wpk[:, 7]]]
            ident = wpk[:, 8]
            qb = spk[:, 0:1]
            kb = spk[:, 1:2]
            gpk = spk[:, 2:51]
            fb = [spk[:, 51:52], spk[:, 52:53]]
            # ---- warmup: ramp the PE p-state and load the ACT table while
            # the input DMAs are in flight (scratch data, results unused) ----
            warm = wpool.tile([128, 64], bf16, tag="warm", name="warm")
            nc.vector.memset(warm[:], 0.0)
            wps = psA.tile([128, 512], f32, tag="ps", name="warmps")
            for i in range(40):
                nc.tensor.matmul(wps[0:64, 0:64], warm[:], warm[:],
                                 start=True, stop=True)
            wact = wpool.tile([128, 1], f32, tag="wact", name="wact")
            nc.scalar.activation(wact[:], warm[:, 0:1], Exp, bias=0.0,
                                 scale=1.0)

            # ---- qm (center rows) first: it gates qmc7 and every P ----
            qm = kq.tile([128, RH, WP], bf16, tag="qm", name="qm")
            qm_f = qm[:].rearrange("p r w -> p (r w)")
            NQM = RH * WP  # 868
            for off in range(0, NQM, 496):
                n = min(496, NQM - off)
                ps = psA.tile([128, 512], f32, tag="ps", name="ps")
                for c2 in range(2):
                    rhs = xv[c2][:].rearrange("p r w -> p (r w)")[:, PAD * WP + off:
                                                                 PAD * WP + off + n]
                    nc.tensor.matmul(ps[:, :n], wq[c2], rhs,
                                     start=(c2 == 0), stop=(c2 == 1))
                nc.scalar.activation(qm_f[:, off:off + n], ps[:, :n], Ident,
                                     bias=qb, scale=1.0)
            km = kq.tile([128, RHP, WP], bf16, tag="km", name="km")
            km_f = km[:].rearrange("p r w -> p (r w)")
            NKM = RHP * WP  # 1240
            for off in range(0, NKM, 496):
                n = min(496, NKM - off)
                ps = psA.tile([128, 512], f32, tag="ps", name="ps")
                for c2 in range(2):
                    rhs = xv[c2][:].rearrange("p r w -> p (r w)")[:, off:off + n]
                    nc.tensor.matmul(ps[:, :n], wk[c2], rhs,
                                     start=(c2 == 0), stop=(c2 == 1))
                nc.scalar.activation(km_f[:, off:off + n], ps[:, :n], Ident,
                                     bias=kb, scale=1.0)
            qmc = qm[:, :, PAD:PAD + W]   # [128, 14, 56]
            # one-time: qm center replicated over kx (kx innermost) for the
            # single-op fused P multiply
            qmc7 = kq.tile([128, RH, W, K], bf16, tag="qmc7", name="qmc7")
            for (k0, k1), eng in (((0, 2), nc.gpsimd), ((2, 4), nc.vector),
                                  ((4, 6), nc.gpsimd), ((6, 7), nc.vector)):
                qmc_b = qmc.copy()
                qmc_b.ap = bass_rust.VecI64Pair(
                    [[RH * WP, 128], [WP, RH], [1, W], [0, k1 - k0]])
                qmc_b.offset = PAD
                if eng is nc.scalar:
                    eng.copy(qmc7[:, :, :, k0:k1], qmc_b)
                else:
                    eng.tensor_copy(qmc7[:, :, :, k0:k1], qmc_b)

            pre = [outp.tile([128, RH, W], bf16, tag=f"pre{c2}", name=f"pre{c2}")
                   for c2 in range(2)]
            t6 = [outp.tile([128, RH, W], bf16, tag=f"t6{c2}", name=f"t6{c2}")
                  for c2 in range(2)]

            # ---- main loop: phase 1 per ky computes P, E, d (score side);
            # phase 2 per (ky, c2) does PV -> S (PE) -> T = S/d (value side).
            # In phased mode all score work is emitted first so the event-
            # driven engines stay leveled; E/d tiles persist per ky. ----
            Es, ds = [], []

            def score_phase(ky):
                P = pp.tile([128, RH, W, K], bf16, tag="P", name="P")
                kv = win_view(km, ky)
                nc.vector.tensor_mul(P[:, :, :, 0:4], kv[:, :, :, 0:4],
                                     qmc7[:, :, :, 0:4])
                nc.vector.tensor_mul(P[:, :, :, 4:K], kv[:, :, :, 4:K],
                                     qmc7[:, :, :, 4:K])
                E = ew.tile([128, RH, W, K], bf16, tag="E", name=f"E{ky}")
                for kx in range(K):
                    j = ky * K + kx
                    nc.scalar.activation(E[:, :, :, kx], P[:, :, :, kx], Exp,
                                         bias=spk[:, 2 + j:3 + j], scale=1.0)
                r = dd.tile([128, RH, W], f32, tag="r", name=f"r{ky}")
                dps = psD.tile([128, 2, 512], f32, tag="dps", name="dps")
                for kx in range(K):
                    nc.tensor.matmul(dps[:, 0, 0:HL], ident,
                                     E[:, 0:RH // 2, :, kx],
                                     start=(kx == 0), stop=(kx == K - 1))
                    nc.tensor.matmul(dps[:, 1, 0:HL], ident,
                                     E[:, RH // 2:RH, :, kx],
                                     start=(kx == 0), stop=(kx == K - 1))
                r_f = r[:].rearrange("p r w -> p (r w)")
                nc.vector.reciprocal_approx_fast(
                    r_f[:].rearrange("p (h x) -> p h x", h=2),
                    dps[:, :, 0:HL])
                return E, r

            def value_phase(ky, c2, E, r):
                r_f = r[:].rearrange("p r w -> p (r w)")
                PV = pvp.tile([128, RH, W, K], bf16, tag=f"PV{c2}",
                              name=f"PV{c2}")
                xvv = win_view(xv[c2], ky)
                peng = nc.vector if (c2 == 0 or ky == K - 1) else nc.gpsimd
                peng.tensor_mul(PV[:, :, :, 0:4], E[:, :, :, 0:4],
                                xvv[:, :, :, 0:4])
                nc.gpsimd.tensor_mul(PV[:, :, :, 4:K], E[:, :, :, 4:K],
                                     xvv[:, :, :, 4:K])
                sp = psS.tile([128, 2, 512], f32, tag="sp", name="sp")
                for kx in range(K):
                    nc.tensor.matmul(sp[:, 0, 0:HL], ident,
                                     PV[:, 0:RH // 2, :, kx],
                                     start=(kx == 0), stop=(kx == K - 1))
                    nc.tensor.matmul(sp[:, 1, 0:HL], ident,
                                     PV[:, RH // 2:RH, :, kx],
                                     start=(kx == 0), stop=(kx == K - 1))
                dst = pre[c2] if ky == 0 else (
                    t6[c2] if ky == K - 1 else
                    sm.tile([128, RH, W], bf16, tag=f"T{c2}", name=f"T{c2}"))
                dst_f = dst[:].rearrange("p r w -> p (r w)")
                nc.vector.tensor_mul(
                    dst_f[:].rearrange("p (h x) -> p h x", h=2),
                    sp[:, :, 0:HL],
                    r_f[:].rearrange("p (h x) -> p h x", h=2))
                if 0 < ky < K - 1:
                    nc.gpsimd.tensor_add(pre[c2][:], pre[c2][:], dst[:])

            if cfg.get("phased"):
                for ky in range(K):
                    Es_ds = score_phase(ky)
                    Es.append(Es_ds[0]); ds.append(Es_ds[1])
                for ky in range(K):
                    for c2 in range(2):
                        value_phase(ky, c2, Es[ky], ds[ky])
            else:
                for ky in range(K):
                    E, d = score_phase(ky)
                    for c2 in range(2):
                        value_phase(ky, c2, E, d)

            # ---- final 1x1 conv: psum accumulates fw^T pre (ky0..5, can
            # run during ky6) then fw^T t6 closes it ----
            for o in range(2):
                y_sb = outp.tile([128, RH, W], f32, tag=f"y{o}", name=f"y{o}")
                y_f = y_sb[:].rearrange("p r w -> p (r w)")
                for h in range(2):
                    ps = psA.tile([128, 512], f32, tag="ps", name="psf")
                    for ci in range(2):
                        rhs = pre[ci][:].rearrange("p r w -> p (r w)")[
                            :, h * HL:(h + 1) * HL]
                        nc.tensor.matmul(ps[:, :HL], fw[ci][o], rhs,
                                         start=(ci == 0), stop=False)
                    for ci in range(2):
                        rhs = t6[ci][:].rearrange("p r w -> p (r w)")[
                            :, h * HL:(h + 1) * HL]
                        nc.tensor.matmul(ps[:, :HL], fw[ci][o], rhs,
                                         start=False, stop=(ci == 1))
                    nc.scalar.activation(y_f[:, h * HL:(h + 1) * HL],
                                         ps[:, :HL], Ident,
                                         bias=fb[o], scale=1.0)
                    nc.sync.dma_start(y_d.ap()[o][:, h * HL:(h + 1) * HL],
                                      y_f[:, h * HL:(h + 1) * HL])

    nc.compile()
    return nc
